# revision 17
# baseline (speedup 1.0000x reference)
"""Trainium2 Bass kernel for nn_Bottleneck_CSA_ConvBlock.

Computation (per image, C=64, H=W=160):
    y  = silu(bn1(conv3x3(x, w1)))
    fv = conv3x3(y, wv)
    k_sum = fk.sum(ch, h); f_scores[c] = scale * sum_hw fq[c,h,w]*k_sum[w]
    scores = softmax_c(f_scores)
    out = x + relu(bn2(scores*fv + y))
(fq/fk never materialize: f_scores reduces to functionals of y's column
sums, as in the reference's algebraic reorder.)

Row-parity full-array conv: each image's rows split into
  A[j]  = padded row 2j    on SBUF partitions 0..63   (channel = partition)
  Bs[j] = padded row 2j-1  on partitions 64..127
so a conv is 6 matmuls per 3-pair sub-block with 128x128 stationary
weights (contraction = 64ch x {A,Bs}, outputs = 64ch x {E=even, O=odd}):
  m1 @pair j   : [A->E]=w_dy1 [A->O]=w_dy0 [Bs->E]=w_dy0 [Bs->O]=0
  m2 @pair j+1 : [A->E]=0     [A->O]=w_dy2 [Bs->E]=w_dy2 [Bs->O]=w_dy1
(x3 column shifts dx) -> 75% PE utilization; images run serially.

Pass 2 folds the whole epilogue core into the conv: scores scale per
OUTPUT channel = per lhsT column, and "+ y" is an identity tap at
(dy=1,dx=1), so conv(y, s[c]*wv + delta) = scores*fv + y. The scaled
weights are built on-device per image (rank-1 scores matmul + 7 small
DVE ops). Epilogue per 4-bank PSUM supertile: one bn2+relu activation,
two adds of x (parity-aligned from SBUF), one bf16 DMA out.

x ships once as bf16 in the parity-padded layout (pads zeroed on host);
the residual reuses it from SBUF. Output leaves as bf16 parity layout,
de-interleaved and upcast on host.

Sharding: pure data parallelism, 2 images per core across 8 cores.
"""

import numpy as np

C = 64
H = W = 160
WP = 162            # padded row width
NJ = 82             # pair slots j=0..81
PIMG2 = NJ * WP     # 13284 per image per partition
NOUT = 81 * W       # 12960 out elements per image per partition
IMG = H * W
BN_EPS = 1e-5

# supertile groups: (j0, n_subblocks); sub-block = 3 pairs, 480 free,
# written at 512-f32 stride inside a [128, 2048] PSUM tile (bank-aligned)
GROUPS = [(0, 4), (12, 4), (24, 4), (36, 4), (48, 4), (60, 4), (72, 3)]

_CACHED = {}


def _build_nc(loop_n=0):
    from contextlib import ExitStack
    import concourse.tile as tile
    from concourse import bacc, mybir
    from concourse.masks import make_identity

    dt = mybir.dt
    AF = mybir.ActivationFunctionType
    AX = mybir.AxisListType
    f32 = dt.float32
    bf16 = dt.bfloat16

    nc = bacc.Bacc("TRN2", target_bir_lowering=False, debug=False, num_devices=8)

    xt_d = nc.dram_tensor("xt", [128, 2, PIMG2], bf16, kind="ExternalInput")
    wp1_d = nc.dram_tensor("wp1", [128, 6, 128], bf16, kind="ExternalInput")
    wpv_d = nc.dram_tensor("wpv", [128, 6, 128], bf16, kind="ExternalInput")
    wdel_d = nc.dram_tensor("wdel", [128, 6, 128], bf16, kind="ExternalInput")
    wq_d = nc.dram_tensor("wqt", [64, 9, 65], bf16, kind="ExternalInput")
    ids_d = nc.dram_tensor("ids", [128, 64], bf16, kind="ExternalInput")
    bn1s_d = nc.dram_tensor("bn1s", [128, 1], f32, kind="ExternalInput")
    bn1b_d = nc.dram_tensor("bn1b", [128, 1], f32, kind="ExternalInput")
    bn2s_d = nc.dram_tensor("bn2s", [128, 1], f32, kind="ExternalInput")
    bn2b_d = nc.dram_tensor("bn2b", [128, 1], f32, kind="ExternalInput")
    out_d = nc.dram_tensor("out", [128, 2, NOUT], bf16, kind="ExternalOutput")

    with tile.TileContext(nc) as tc:
        ctx_lp = nc.allow_low_precision("bf16 matmul path; fp32 PSUM accumulation")
        ctx_lp.__enter__()
        _stk = ExitStack()
        if loop_n:
            _stk.enter_context(tc.For_i(0, loop_n, 1))
        with (
            tc.tile_pool(name="const", bufs=1) as const,
            tc.tile_pool(name="pers", bufs=1) as pers,
            tc.tile_pool(name="small", bufs=1) as small,
        ):
            wp1_sb = const.tile([128, 6, 128], bf16)
            nc.sync.dma_start(out=wp1_sb[:], in_=wp1_d.ap())
            wpv_sb = const.tile([128, 6, 128], bf16)
            nc.sync.dma_start(out=wpv_sb[:], in_=wpv_d.ap())
            wdel_sb = const.tile([128, 6, 128], bf16)
            nc.sync.dma_start(out=wdel_sb[:], in_=wdel_d.ap())
            wq_sb = const.tile([64, 9, 65], bf16)
            nc.sync.dma_start(out=wq_sb[:], in_=wq_d.ap())
            ids_sb = const.tile([128, 64], bf16)
            nc.sync.dma_start(out=ids_sb[:], in_=ids_d.ap())
            bn1s = const.tile([128, 1], f32)
            nc.sync.dma_start(out=bn1s[:], in_=bn1s_d.ap())
            bn1b = const.tile([128, 1], f32)
            nc.sync.dma_start(out=bn1b[:], in_=bn1b_d.ap())
            bn2s = const.tile([128, 1], f32)
            nc.sync.dma_start(out=bn2s[:], in_=bn2s_d.ap())
            bn2b = const.tile([128, 1], f32)
            nc.sync.dma_start(out=bn2b[:], in_=bn2b_d.ap())
            ident = const.tile([128, 128], f32)
            make_identity(nc, ident[:])
            ones_sb = const.tile([128, 128], bf16)
            nc.vector.memset(ones_sb[:], 1.0)

            xT = pers.tile([128, 2, PIMG2 + 4], bf16)
            for i in range(2):
                for a, b in ((0, 28), (28, 55), (55, 82)):
                    nc.sync.dma_start(out=xT[:, i, a * WP:b * WP],
                                      in_=xt_d.ap()[:, i, a * WP:b * WP])

            yT = pers.tile([128, 2, PIMG2 + 4], bf16)
            for i in range(2):
                y3 = yT[:, i, 0:PIMG2].rearrange("p (j c) -> p j c", c=WP)
                nc.vector.memset(yT[:, i, 0:WP], 0.0)
                nc.vector.memset(yT[:, i, (NJ - 1) * WP:NJ * WP], 0.0)
                nc.vector.memset(y3[:, 1:NJ - 1, 0:1], 0.0)
                nc.vector.memset(y3[:, 1:NJ - 1, WP - 1:WP], 0.0)

            cacc0 = small.tile([128, WP], f32, tag="cacc0")
            cacc1 = small.tile([128, WP], f32, tag="cacc1")
            caccs = [cacc0, cacc1]

            def rhs_view(src, i, pair0, dx, nsub):
                base = pair0 * WP + dx
                return src[:, i, base:base + 3 * nsub * WP].rearrange(
                    "p (j c) -> p j c", c=WP)[:, :, 0:W]

            def conv_group(src, i, j0, nsub, wsb, ps):
                for k6 in range(6):
                    dx, m = k6 // 2, k6 % 2
                    for s in range(nsub):
                        rhs = rhs_view(src, i, j0 + 3 * s + m, dx, 1)
                        nc.tensor.matmul(
                            ps[:, s * 512:s * 512 + 480], wsb[:, 2 * dx + m, :],
                            rhs, start=(k6 == 0), stop=(k6 == 5),
                        )

            def ps_in4(ps, lo, nsub, plo=0, phi=128):
                # PSUM supertile as [p, nsub, 3, 160] starting at sub-block lo
                return ps[plo:phi, lo * 512:(lo + nsub) * 512].rearrange(
                    "p (g e) -> p g e", e=512)[:, :, 0:480].rearrange(
                    "p g (j c) -> p g j c", c=W)

            def y_out4(dst, i, pair0, nsub, plo, phi):
                base = pair0 * WP
                return dst[plo:phi, i, base:base + 3 * nsub * WP].rearrange(
                    "p (g j c) -> p g j c", j=3, c=WP)[:, :, :, 1:1 + W]

            # ---------------- pass 1: conv1 -> yT (+ colsum partials) --------
            with tc.tile_pool(name="ps1", bufs=2, space="PSUM") as ps1:
                for i in range(2):
                    cacc = caccs[i]
                    pend = []

                    def p1_epi(i_, j0, nsub, ps):
                        y3 = yT[:, i_, 0:PIMG2].rearrange(
                            "p (j c) -> p j c", c=WP)
                        if j0 == 0:
                            # E: pairs 1..2 from sub 0, pairs 3.. from subs 1..
                            nc.scalar.activation(
                                out=y3[0:64, 1:3, 1:1 + W],
                                in_=ps[0:64, W:480].rearrange(
                                    "p (j c) -> p j c", c=W),
                                func=AF.Silu, bias=bn1b[0:64], scale=bn1s[0:64])
                            nc.scalar.activation(
                                out=y_out4(yT, i_, 3, nsub - 1, 0, 64),
                                in_=ps_in4(ps, 1, nsub - 1, 0, 64),
                                func=AF.Silu, bias=bn1b[0:64], scale=bn1s[0:64])
                        else:
                            nc.scalar.activation(
                                out=y_out4(yT, i_, j0, nsub, 0, 64),
                                in_=ps_in4(ps, 0, nsub, 0, 64),
                                func=AF.Silu, bias=bn1b[0:64], scale=bn1s[0:64])
                        if j0 == 72:
                            # O: Bs pairs 73..80 (last sub-block only 2 pairs)
                            nc.scalar.activation(
                                out=y_out4(yT, i_, 73, nsub - 1, 64, 128),
                                in_=ps_in4(ps, 0, nsub - 1, 64, 128),
                                func=AF.Silu, bias=bn1b[64:128],
                                scale=bn1s[64:128])
                            nc.scalar.activation(
                                out=y3[64:128, 79:81, 1:1 + W],
                                in_=ps[64:128, 2 * 512:2 * 512 + 2 * W].rearrange(
                                    "p (j c) -> p j c", c=W),
                                func=AF.Silu, bias=bn1b[64:128],
                                scale=bn1s[64:128])
                        else:
                            nc.scalar.activation(
                                out=y_out4(yT, i_, j0 + 1, nsub, 64, 128),
                                in_=ps_in4(ps, 0, nsub, 64, 128),
                                func=AF.Silu, bias=bn1b[64:128],
                                scale=bn1s[64:128])
                        # colsum partial: disjoint pair ranges [j0, j0+3n)
                        # tile [0, 81); each Bs slot written by group g lands
                        # in group g or g+1's range, always after its write.
                        lo = j0
                        hi = j0 + 3 * nsub
                        part = small.tile([128, WP], f32, tag=f"part{i_}")
                        nc.vector.reduce_sum(
                            part[:],
                            yT[:, i_, lo * WP:hi * WP].rearrange(
                                "p (j c) -> p c j", c=WP),
                            axis=AX.X)
                        if j0 == 0:
                            nc.vector.tensor_copy(cacc[:], part[:])
                        else:
                            nc.vector.tensor_add(cacc[:], cacc[:], part[:])

                    for j0, nsub in GROUPS:
                        ps = ps1.tile([128, 2048], f32, tag="ps")
                        conv_group(xT, i, j0, nsub, wp1_sb, ps)
                        for fn, args in pend:
                            fn(*args)
                        pend = [(p1_epi, (i, j0, nsub, ps))]
                    for fn, args in pend:
                        fn(*args)

            # ------------- scores + scaled conv_v weights (per image) --------
            with tc.tile_pool(name="pss", bufs=1, space="PSUM") as pss:
                wv_im0 = small.tile([128, 6, 128], bf16, tag="wvim0")
                wv_im1 = small.tile([128, 6, 128], bf16, tag="wvim1")
                wv_im = [wv_im0, wv_im1]
                for i in range(2):
                    csb = small.tile([128, WP], bf16, tag=f"csb{i}")
                    nc.vector.tensor_copy(csb[:], caccs[i][:])
                    C_ps = pss.tile([64, WP], f32, tag="cps")
                    nc.tensor.matmul(C_ps[:], ids_sb[:, :], csb[:],
                                     start=True, stop=True)
                    r1_ps = pss.tile([64, WP], f32, tag="r1")
                    nc.tensor.matmul(r1_ps[:], ids_sb[64:128, :],
                                     yT[64:128, i, WP:2 * WP],
                                     start=True, stop=True)
                    Cb = small.tile([64, WP], bf16, tag=f"Cb{i}")
                    nc.vector.tensor_copy(Cb[:], C_ps[:])
                    r1b = small.tile([64, WP], bf16, tag=f"r1b{i}")
                    nc.vector.tensor_copy(r1b[:], r1_ps[:])
                    CmL = small.tile([64, WP], bf16, tag=f"CmL{i}")
                    nc.vector.tensor_sub(CmL[:], Cb[:],
                                         yT[0:64, i, 80 * WP:81 * WP])
                    CmF = small.tile([64, WP], bf16, tag=f"CmF{i}")
                    nc.vector.tensor_sub(CmF[:], Cb[:], r1b[:])
                    s_of = {0: CmL, 1: Cb, 2: CmF}

                    qp = pss.tile([65, W], f32, tag="qp")
                    for k9 in range(9):
                        dy, dx = divmod(k9, 3)
                        nc.tensor.matmul(
                            qp[:], wq_sb[:, k9, :], s_of[dy][:, dx:dx + W],
                            start=(k9 == 0), stop=(k9 == 8),
                        )
                    q_sb = small.tile([65, W], bf16, tag=f"q{i}")
                    nc.vector.tensor_copy(q_sb[:], qp[:])
                    bc = pss.tile([64, W], f32, tag="bc")
                    nc.tensor.matmul(bc[:], ones_sb[64:65, 0:64], q_sb[64:65, :],
                                     start=True, stop=True)
                    t_sb = small.tile([64, W], f32, tag=f"t{i}")
                    nc.vector.tensor_mul(t_sb[:], q_sb[0:64, :], bc[:])
                    fs = small.tile([64, 1], f32, tag=f"fs{i}")
                    nc.vector.reduce_sum(fs[:], t_sb[:], axis=AX.X)
                    tr = pss.tile([1, 64], f32, tag="tr")
                    nc.tensor.transpose(tr[:], fs[:], ident[0:64, 0:64])
                    frow = small.tile([1, 64], f32, tag=f"fr{i}")
                    nc.vector.tensor_copy(frow[:], tr[:])
                    mx = small.tile([1, 1], f32, tag=f"mx{i}")
                    sm = small.tile([1, 1], f32, tag=f"sm{i}")
                    rs = small.tile([1, 1], f32, tag=f"rs{i}")
                    srow = small.tile([1, 64], f32, tag=f"sr{i}")
                    nc.vector.reduce_max(mx[:], frow[:], axis=AX.X, negate=True)
                    nc.scalar.activation(out=srow[:], in_=frow[:], func=AF.Exp,
                                         bias=mx[:], scale=1.0)
                    nc.vector.reduce_sum(sm[:], srow[:], axis=AX.X)
                    nc.vector.reciprocal(rs[:], sm[:])
                    nc.vector.tensor_scalar_mul(srow[:], srow[:], rs[:])
                    srow2 = small.tile([1, 128], bf16, tag=f"sr2{i}")
                    nc.vector.tensor_copy(srow2[0:1, 0:64], srow[:])
                    nc.vector.tensor_copy(srow2[0:1, 64:128], srow[:])
                    S_ps = pss.tile([128, 128], f32, tag="S")
                    nc.tensor.matmul(S_ps[:], ones_sb[0:1, :], srow2[:],
                                     start=True, stop=True)
                    S_b = small.tile([128, 128], bf16, tag=f"Sb{i}")
                    nc.vector.tensor_copy(S_b[:], S_ps[:])
                    for k6 in range(6):
                        nc.vector.tensor_mul(wv_im[i][:, k6, :],
                                             wpv_sb[:, k6, :], S_b[:])
                    nc.vector.tensor_add(wv_im[i][:], wv_im[i][:], wdel_sb[:])

            # ---------------- pass 2: conv_v(scaled)+y -> epilogue -> out ----
            with (
                tc.tile_pool(name="ps2", bufs=2, space="PSUM") as ps2,
                tc.tile_pool(name="epi", bufs=2) as epi,
            ):
                for i in range(2):
                    x3 = xT[:, i, 0:PIMG2].rearrange("p (j c) -> p j c", c=WP)
                    pend = []

                    def p2_epi(i_, j0, nsub, ps):
                        n = nsub * 3 * W
                        rt = epi.tile([128, 12 * W], bf16, tag="rt")
                        rt4 = rt[:, 0:n].rearrange("p (g j c) -> p g j c",
                                                   j=3, c=W)
                        nc.scalar.activation(
                            out=rt4, in_=ps_in4(ps, 0, nsub),
                            func=AF.Relu, bias=bn2b[:], scale=bn2s[:])
                        ot = epi.tile([128, 12 * W], bf16, tag="ot")
                        ot4 = ot[:, 0:n].rearrange("p (g j c) -> p g j c",
                                                   j=3, c=W)
                        nc.vector.tensor_add(
                            ot4[0:64], rt4[0:64],
                            y_out4(xT, i_, j0, nsub, 0, 64))
                        nc.vector.tensor_add(
                            ot4[64:128], rt4[64:128],
                            y_out4(xT, i_, j0 + 1, nsub, 64, 128))
                        oap = out_d.ap()
                        if j0 == 0:
                            nc.sync.dma_start(out=oap[0:64, i_, W:12 * W],
                                              in_=ot[0:64, W:12 * W])
                            nc.sync.dma_start(out=oap[64:128, i_, 0:12 * W],
                                              in_=ot[64:128, 0:12 * W])
                        elif j0 == 72:
                            nc.sync.dma_start(
                                out=oap[0:64, i_, 72 * W:81 * W],
                                in_=ot[0:64, 0:9 * W])
                            nc.sync.dma_start(
                                out=oap[64:128, i_, 72 * W:80 * W],
                                in_=ot[64:128, 0:8 * W])
                        else:
                            nc.sync.dma_start(
                                out=oap[:, i_, j0 * W:(j0 + 12) * W],
                                in_=ot[:, :])

                    for j0, nsub in GROUPS:
                        ps = ps2.tile([128, 2048], f32, tag="ps")
                        conv_group(yT, i, j0, nsub, wv_im[i], ps)
                        for fn, args in pend:
                            fn(*args)
                        pend = [(p2_epi, (i, j0, nsub, ps))]
                    for fn, args in pend:
                        fn(*args)
        _stk.close()
        ctx_lp.__exit__(None, None, None)
    nc.compile()
    return nc


def _get_nc():
    if "nc" not in _CACHED:
        _CACHED["nc"] = _build_nc()
    return _CACHED["nc"]


def _prep_weights(w_cv1, wq, wk, wv, g1, b1, m1, v1, g2, b2, m2, v2):
    import ml_dtypes
    bf = ml_dtypes.bfloat16

    def wpar(w):
        """[co, ci, ky, kx] -> [128, 6, 128] parity-block lhsTs."""
        out = np.zeros((128, 6, 128), np.float32)

        def wT(dy, dx):   # [ci, co]
            return w[:, :, dy, dx].T

        for dx in range(3):
            m1b = out[:, 2 * dx + 0, :]
            m1b[0:64, 0:64] = wT(1, dx)
            m1b[0:64, 64:128] = wT(0, dx)
            m1b[64:128, 0:64] = wT(0, dx)
            m2b = out[:, 2 * dx + 1, :]
            m2b[0:64, 64:128] = wT(2, dx)
            m2b[64:128, 0:64] = wT(2, dx)
            m2b[64:128, 64:128] = wT(1, dx)
        return np.ascontiguousarray(out.astype(bf))

    # identity tap at (dy=1, dx=1): k=2 [A->E] diag, k=3 [Bs->O] diag
    wdel = np.zeros((128, 6, 128), np.float32)
    wdel[0:64, 2, 0:64] = np.eye(C)
    wdel[64:128, 3, 64:128] = np.eye(C)
    wdel = np.ascontiguousarray(wdel.astype(bf))

    scale = 1.0 / (float(W) ** 0.5 * float(H) * float(H))
    q = wq.transpose(1, 2, 3, 0).reshape(C, 9, C) * scale    # [j, 9, c]
    ks = wk.sum(axis=0).reshape(C, 9, 1)                     # [j, 9, 1]
    wqt = np.ascontiguousarray(
        np.concatenate([q, ks], axis=2).astype(bf))          # [64, 9, 65]

    ids = np.ascontiguousarray(
        np.concatenate([np.eye(C), np.eye(C)], axis=0).astype(bf))

    s1 = (g1 / np.sqrt(v1 + BN_EPS)).astype(np.float32)
    b1p = (b1 - m1 * s1).astype(np.float32)
    s2 = (g2 / np.sqrt(v2 + BN_EPS)).astype(np.float32)
    b2p = (b2 - m2 * s2).astype(np.float32)

    def dup(v):
        return np.ascontiguousarray(
            np.concatenate([v, v]).reshape(128, 1).astype(np.float32))

    return dict(wp1=wpar(w_cv1), wpv=wpar(wv), wdel=wdel, wqt=wqt, ids=ids,
                bn1s=dup(s1), bn1b=dup(b1p), bn2s=dup(s2), bn2b=dup(b2p))


def _make_core_inputs(x2):
    """Per-core input map from this core's 2 images [2, C, H, W] f32.

    Parity-padded bf16 layout:
      partition p<64 : A[j]  = padded row 2j   (orig row 2j-1), channel p
      partition p>=64: Bs[j] = padded row 2j-1 (orig row 2j-2), channel p-64
    with all pads zero.
    """
    import ml_dtypes
    xt = np.zeros((128, 2, NJ, WP), dtype=ml_dtypes.bfloat16)
    for i in range(2):
        img = x2[i]                       # [64, 160, 160]
        xt[0:64, i, 1:81, 1:1 + W] = img[:, 1::2, :]
        xt[64:128, i, 1:81, 1:1 + W] = img[:, 0::2, :]
    return {"xt": np.ascontiguousarray(xt.reshape(128, 2, PIMG2))}


def _unpack_out(res_out):
    """[128, 2, NOUT] bf16 parity layout -> [2, C, H, W] f32."""
    r = np.asarray(res_out, np.float32).reshape(128, 2, 81, W)
    y = np.empty((2, C, H, W), np.float32)
    for i in range(2):
        y[i, :, 1::2, :] = r[0:64, i, 1:81]    # E[j] = orig row 2j-1
        y[i, :, 0::2, :] = r[64:128, i, 0:80]  # O[j] = orig row 2j
    return y


def _ensure_axon_devices():
    """Make sure jax can see the 8 axon-tunneled NeuronCores even if the
    calling process pinned JAX_PLATFORMS=cpu before importing us."""
    import os
    envp = os.environ.get("JAX_PLATFORMS", "")
    if envp and "axon" not in envp:
        os.environ.pop("JAX_PLATFORMS", None)
    import jax
    try:
        devs = jax.devices()
        if len(devs) >= 8 and all("cpu" not in str(d).lower() for d in devs[:8]):
            return
    except Exception:
        pass
    try:
        from jax._src import xla_bridge
        xla_bridge.backends.cache_clear()
    except Exception:
        pass
    try:
        import jax.extend.backend as jeb
        jeb.clear_backends()
    except Exception:
        pass


def kernel(x, w_cv1, g1, b1, m1, v1, wq, wk, wv, g2, b2, m2, v2):
    _ensure_axon_devices()
    from concourse.bass_utils import run_bass_kernel_spmd

    x = np.asarray(x, dtype=np.float32)
    consts = _prep_weights(
        np.asarray(w_cv1, np.float32), np.asarray(wq, np.float32),
        np.asarray(wk, np.float32), np.asarray(wv, np.float32),
        np.asarray(g1, np.float32), np.asarray(b1, np.float32),
        np.asarray(m1, np.float32), np.asarray(v1, np.float32),
        np.asarray(g2, np.float32), np.asarray(b2, np.float32),
        np.asarray(m2, np.float32), np.asarray(v2, np.float32))
    nc = _get_nc()
    in_maps = []
    for i in range(8):
        m = _make_core_inputs(x[2 * i:2 * i + 2])
        m.update(consts)
        in_maps.append(m)
    res = run_bass_kernel_spmd(nc, in_maps, core_ids=list(range(8)))
    outs = [_unpack_out(r["out"]) for r in res.results]
    return np.concatenate(outs, axis=0).astype(np.float32)
